# revision 23
# baseline (speedup 1.0000x reference)
"""Trainium2 Bass kernel for nn_MetaRouter (dense_transformer).

Contract: kernel(**inputs) takes FULL unsharded inputs (as produced by
reference.setup_inputs()) and returns the FULL [B, D] logits, matching
reference.reference(**inputs).

Strategy (v2):
  - Data-parallel over batch: B=16 split as 2 batches per core x 8 cores.
    All parameters replicated. No collectives.
  - Host side: tokens with attention_mask==0 receive softmax weight exactly
    0 for every query, so each batch row is compacted to its unmasked
    tokens (padded to a multiple of 128; pad slots get a -1e9 score bias).
    The compacted token stream is cast to bf16 AND pre-transposed on the
    host into [128 feat-partition, tile, chunk, token] layout so the device
    runs ZERO input transposes (v1 spent ~50us of PE time on them).
  - Scores are folded into the projection matmul: for LayerNorm'd x,
        score[t,q] = rstd[t] * (ts[t,:] @ WM[:,q])  + pb[t]
    with WM = W_proj@(g*q) - (W_proj@1_H) * (sum(g*q)/H)  precomputed on
    the host (the mean-correction term is exact).  So each k-chunk issues
    one N=512 matmul (projection) plus one N=32 matmul (scores) sharing
    the same stationary tile.
  - Softmax denominator is never computed: LayerNorm(ctx) is invariant
    under per-row positive scaling and constant shifts, so the context
    matmul consumes unnormalized weights e' = exp(score)*rstd and
    UN-centered, UN-scaled projections x_raw:
        ctx_raw = sum_t e'[q,t] * px[t,:]  =  S*(ctx_true + kappa*1)
    which LayerNorms to exactly LN(ctx_true).
  - rstd = 1/sqrt(var) computed on the vector engine with Newton
    iterations (no ACT Sqrt -> no activation-table thrashing between Exp
    and Sqrt; ACT only loads Exp + Gelu tables).
  - FFN tail identical in structure to v1 (PE transposes of the tiny
    [32,512] context, two gelu matmul layers, temperature folded on host).
"""

import os

import numpy as np
import ml_dtypes

import concourse.bass as bass
import concourse.bacc as bacc
import concourse.tile as tile
from concourse import mybir
from concourse.masks import make_identity

P = 128
H = 512
TOKD = 4096
KC = TOKD // P  # 32 k-chunks of the projection contraction
NQ = 32         # 17 queries (1 global + 16 domains) padded to 32
D = 16
B = 16
S = 2048
N_CORES = 8
B_LOCAL = B // N_CORES
EPS = 1e-5
F32 = mybir.dt.float32
BF16 = mybir.dt.bfloat16
U32 = mybir.dt.uint32
NS = H + NQ  # projection + score columns

NEWTON_MODE = os.environ.get("KERNEL_NEWTON", "bithack")  # bithack | act
# fp8e4m3 DoubleRow was tried and runs ~1.3x faster on the PE, but its
# ~4-5% per-product quantization error propagates to ~3.3e-2 relative
# error on the logits (gate: 2e-2) -- attention averaging does not shrink
# it. Keep bf16.
FP8 = os.environ.get("KERNEL_FP8", "0") == "1"
FP8DT = mybir.dt.float8e4
TS_DT = FP8DT if FP8 else BF16
# LayerNorm makes the projection scale-invariant, so the weights can be
# pre-scaled into fp8e4m3's sweet spot (w ~ N(0, 0.02) -> ~N(0, 1.3)).
WSCALE = 64.0 if FP8 else 1.0


def build_nc(S_c: int, b_out_s: float, skip=frozenset()):
    """Build the per-core Bass program for padded/compacted seq length S_c."""
    assert S_c % P == 0
    NT = S_c // P          # token tiles per batch row
    TT = B_LOCAL * NT      # token tiles per core

    nc = bacc.Bacc("TRN2", target_bir_lowering=False, num_swdge_queues=2)

    # host-pretransposed token stream: [p, t, c, tok] = ts[t*128+tok, c*128+p]
    tst = nc.declare_dram_parameter("tst", [P, TT * KC * P], TS_DT, isOutput=False)
    # host layout [p, c, n] = W_ext[c*128+p, n];  W_ext = [W_proj | WM]
    # (fp8: [p, pair, j, n] = W_ext[(2*pair+j)*128+p, n] for DoubleRow rhs)
    wx = nc.declare_dram_parameter("wx", [P, KC * NS], TS_DT, isOutput=False)
    # pad score bias, [p, t] = 0.0 for live tokens / -1e9 for pad slots
    pbt = nc.declare_dram_parameter("pbt", [P, TT], F32, isOutput=False)
    cg = nc.declare_dram_parameter("cg", [NQ, H], F32, isOutput=False)
    cb = nc.declare_dram_parameter("cb", [NQ, H], F32, isOutput=False)
    fg = nc.declare_dram_parameter("fg", [1, H], F32, isOutput=False)
    fb = nc.declare_dram_parameter("fb", [1, H], F32, isOutput=False)
    w1 = nc.declare_dram_parameter("w1", [2 * H, H], BF16, isOutput=False)
    bf1 = nc.declare_dram_parameter("bf1", [1, H], BF16, isOutput=False)
    w2 = nc.declare_dram_parameter("w2", [H, H], BF16, isOutput=False)
    bf2 = nc.declare_dram_parameter("bf2", [1, H], BF16, isOutput=False)
    wo = nc.declare_dram_parameter("wo", [1, H], F32, isOutput=False)
    out = nc.declare_dram_parameter("out", [B_LOCAL, D], F32, isOutput=True)

    with tile.TileContext(nc) as tc:
        _emit(tc, nc, S_c, NT, TT, b_out_s, skip,
              tst=tst, wx=wx, pbt=pbt, cg=cg, cb=cb, fg=fg, fb=fb,
              w1=w1, bf1=bf1, w2=w2, bf2=bf2, wo=wo, out=out)
    nc.compile()
    return nc


def _emit(tc, nc, S_c, NT, TT, b_out_s, skip, *, tst, wx, pbt, cg, cb,
          fg, fb, w1, bf1, w2, bf2, wo, out):
    from contextlib import ExitStack
    ctx = ExitStack()
    with ctx:
        const = ctx.enter_context(tc.tile_pool(name="const", bufs=1))
        tsp = ctx.enter_context(tc.tile_pool(name="tsp", bufs=4))
        xp = ctx.enter_context(tc.tile_pool(name="xp", bufs=1))
        lnp = ctx.enter_context(tc.tile_pool(name="lnp", bufs=4))
        p2 = ctx.enter_context(tc.tile_pool(name="p2", bufs=1))
        psx = ctx.enter_context(tc.tile_pool(name="psx", bufs=2, space="PSUM"))
        pssc = ctx.enter_context(tc.tile_pool(name="pssc", bufs=2, space="PSUM"))
        pctx = ctx.enter_context(tc.tile_pool(name="pctx", bufs=2, space="PSUM"))
        pst = ctx.enter_context(tc.tile_pool(name="pst", bufs=1, space="PSUM"))
        pffn = ctx.enter_context(tc.tile_pool(name="pffn", bufs=1, space="PSUM"))

        # ---- weight + const loads; k-interleaved across the two HWDGE
        # rings so chunk k arrives roughly in consumption order ----
        if FP8:
            w_sb = const.tile([P, KC // 2, 2, NS], TS_DT)
            _wx = wx.ap().rearrange("p (c j n) -> p c j n", c=KC // 2, j=2)
            for _q in range(KC // 2):
                eng = nc.sync if _q % 2 == 0 else nc.scalar
                eng.dma_start(out=w_sb[:, _q:_q + 1, :, :],
                              in_=_wx[:, _q:_q + 1, :, :])
        else:
            w_sb = const.tile([P, KC, NS], TS_DT)
            _wx = wx.ap().rearrange("p (c n) -> p c n", c=KC)
            for _q in range(KC // 2):
                _qs = slice(_q * 2, _q * 2 + 2)
                eng = nc.sync if _q % 2 == 0 else nc.scalar
                eng.dma_start(out=w_sb[:, _qs, :], in_=_wx[:, _qs, :])

        pbt_sb = const.tile([P, TT], F32)
        nc.sync.dma_start(out=pbt_sb, in_=pbt.ap())

        ts_tiles = [None] * TT
        _tst = tst.ap().rearrange("p (t x) -> p t x", t=TT)

        load_engines = [nc.gpsimd, nc.sync, nc.gpsimd, nc.scalar]

        def load(t):
            tile_ = tsp.tile([P, KC, P], TS_DT, tag="ts")
            src = _tst[:, t, :].rearrange("p (c x) -> p c x", c=KC)
            if t < 3:
                # startup: split across the SWDGE queues for parallelism
                for s in range(4):
                    cs = slice(s * (KC // 4), (s + 1) * (KC // 4))
                    nc.gpsimd.dma_start(out=tile_[:, cs, :], in_=src[:, cs, :])
            elif t < 6:
                h = KC // 2
                nc.gpsimd.dma_start(out=tile_[:, :h, :], in_=src[:, :h, :])
                nc.gpsimd.dma_start(out=tile_[:, h:, :], in_=src[:, h:, :])
            else:
                eng = load_engines[t % len(load_engines)]
                eng.dma_start(out=tile_, in_=src)
            ts_tiles[t] = tile_

        PF = 5
        for _t in range(min(PF, TT)):
            load(_t)

        deferred = {}

        def bcast(dram, parts, dt=F32):
            t = const.tile([parts, H], dt, tag=f"c_{dram.name}")
            a = dram.ap()
            nc.scalar.dma_start(
                out=t, in_=bass.AP(tensor=a.tensor, offset=a.offset,
                                   ap=[[0, parts]] + list(a.ap[1:])))
            return t

        def load_p2_consts():
            w1_sb = const.tile([P, 8, H], BF16)
            nc.scalar.dma_start(out=w1_sb,
                                in_=w1.ap().rearrange("(c p) h -> p c h", p=P))
            w2_sb = const.tile([P, 4, H], BF16)
            nc.scalar.dma_start(out=w2_sb,
                                in_=w2.ap().rearrange("(c p) h -> p c h", p=P))
            fg_sb = bcast(fg, NQ)
            fb_sb = bcast(fb, NQ)
            wo_sb = bcast(wo, NQ)
            cg_sb = const.tile([NQ, H], F32)
            nc.scalar.dma_start(out=cg_sb, in_=cg.ap())
            cb_sb = const.tile([NQ, H], F32)
            nc.scalar.dma_start(out=cb_sb, in_=cb.ap())
            bf1_sb = const.tile([1, H], BF16)
            nc.scalar.dma_start(out=bf1_sb, in_=bf1.ap())
            bf2_sb = const.tile([1, H], BF16)
            nc.scalar.dma_start(out=bf2_sb, in_=bf2.ap())
            deferred.update(w1_sb=w1_sb, w2_sb=w2_sb, fg_sb=fg_sb,
                            fb_sb=fb_sb, wo_sb=wo_sb, cg_sb=cg_sb, cb_sb=cb_sb,
                            bf1_sb=bf1_sb, bf2_sb=bf2_sb)

        ones_row = const.tile([1, P], BF16)
        nc.vector.memset(ones_row, 1.0)
        ones_col = const.tile([P, D], BF16)
        nc.vector.memset(ones_col, 1.0)
        id32 = const.tile([NQ, NQ], BF16)
        make_identity(nc, id32)
        magic_u = const.tile([P, max(TT, NQ)], U32)
        nc.vector.memset(magic_u, 0x5f3759df)
        eps_sb = const.tile([P, 1], F32)
        nc.vector.memset(eps_sb, EPS)

        # persistent activations
        x_raw = xp.tile([P, TT, H], BF16)       # un-normalized projections
        logit_sb = xp.tile([NQ, B_LOCAL], F32)

        def newton_rsqrt(out_ap, v_ap, p, n, tag):
            """out = 1/sqrt(v) elementwise, on DVE only (no ACT table).

            Bit-hack seed + 2 Newton steps: ~4e-6 rel error in fp32.
            """
            sh = lnp.tile([p, n], U32, tag=f"nw_sh_{tag}")
            nc.vector.tensor_scalar(out=sh, in0=v_ap.bitcast(U32),
                                    scalar1=1, scalar2=None,
                                    op0=mybir.AluOpType.logical_shift_right)
            y = lnp.tile([p, n], F32, tag=f"nw_y_{tag}")
            nc.vector.tensor_sub(out=y.bitcast(U32), in0=magic_u[:p, :n],
                                 in1=sh)
            t_ = lnp.tile([p, n], F32, tag=f"nw_t_{tag}")
            for it in range(2):
                nc.vector.tensor_mul(out=t_, in0=y, in1=y)
                nc.vector.scalar_tensor_tensor(
                    out=t_, in0=t_, scalar=-0.5, in1=v_ap,
                    op0=mybir.AluOpType.mult, op1=mybir.AluOpType.mult)
                nc.vector.scalar_tensor_tensor(
                    out=(y if it == 0 else out_ap), in0=t_, scalar=1.5, in1=y,
                    op0=mybir.AluOpType.add, op1=mybir.AluOpType.mult)

        def act_rsqrt(out_ap, v_ap, p, n, tag):
            s = lnp.tile([p, n], F32, tag=f"as_{tag}")
            nc.scalar.activation(out=s, in_=v_ap,
                                 func=mybir.ActivationFunctionType.Sqrt,
                                 bias=eps_sb[:p], scale=1.0)
            nc.vector.reciprocal(out=out_ap, in_=s)

        rsqrt = newton_rsqrt if NEWTON_MODE == "bithack" else act_rsqrt

        # ---------------- phase 1: project + scores + ctx per token tile ----
        ctx_ps = [None] * B_LOCAL
        expw_t = [None] * TT

        def ctx_mm(t):
            # emitted one tile late so expw(t) is ready when PE reaches it
            b, i = divmod(t, NT)
            if i == 0:
                pc = pctx.tile([NQ, H], F32, tag="pc")
                ctx_ps[b] = pc
            nc.tensor.matmul(ctx_ps[b], lhsT=expw_t[t], rhs=x_raw[:, t, :],
                             start=(i == 0), stop=(i == NT - 1))
            expw_t[t] = None

        def project(t):
            ts_sb = ts_tiles[t]
            px = psx.tile([P, H], F32, tag="px")
            sc = pssc.tile([P, NQ], F32, tag="sc")
            if FP8:
                DR = mybir.MatmulPerfMode.DoubleRow
                sc_mode = os.environ.get("KERNEL_SC_MODE", "dr")
                for p in range(KC // 2):
                    lt = ts_sb[:, 2 * p:2 * p + 2, :]
                    nc.tensor.matmul(px, lhsT=lt, rhs=w_sb[:, p, :, :H],
                                     start=(p == 0), stop=(p == KC // 2 - 1),
                                     perf_mode=DR)
                    if sc_mode == "dr":
                        nc.tensor.matmul(sc, lhsT=lt, rhs=w_sb[:, p, :, H:],
                                         start=(p == 0),
                                         stop=(p == KC // 2 - 1),
                                         perf_mode=DR)
                if sc_mode == "single":
                    for k in range(KC):
                        nc.tensor.matmul(sc, lhsT=ts_sb[:, k, :],
                                         rhs=w_sb[:, k // 2, k % 2, H:],
                                         start=(k == 0), stop=(k == KC - 1))
                elif sc_mode == "off":
                    nc.vector.memset(sc, 0.0)
            else:
                for k in range(KC):
                    nc.tensor.matmul(px, lhsT=ts_sb[:, k, :],
                                     rhs=w_sb[:, k, :H],
                                     start=(k == 0), stop=(k == KC - 1))
                    nc.tensor.matmul(sc, lhsT=ts_sb[:, k, :],
                                     rhs=w_sb[:, k, H:],
                                     start=(k == 0), stop=(k == KC - 1))
            ts_tiles[t] = None
            if t > 0:
                ctx_mm(t - 1)
            stats = lnp.tile([P, 6], F32, tag="stats")
            nc.vector.bn_stats(out=stats, in_=px)
            mv = lnp.tile([P, 2], F32, tag="mv")
            nc.vector.bn_aggr(out=mv, in_=stats)
            nc.scalar.copy(out=x_raw[:, t, :], in_=px)
            # rstd for this tile's tokens (per-token over H -> tile-local)
            v = lnp.tile([P, 1], F32, tag="ptv")
            nc.vector.tensor_scalar_add(out=v, in0=mv[:, 1:2], scalar1=EPS)
            rstd = lnp.tile([P, 1], F32, tag="ptr")
            rsqrt(rstd, v, P, 1, "p1")
            # unnormalized softmax numerator, rstd folded for the ctx trick
            sexp = lnp.tile([P, NQ], F32, tag="sexp")
            nc.scalar.activation(out=sexp, in_=sc,
                                 func=mybir.ActivationFunctionType.Exp,
                                 bias=pbt_sb[:, t:t + 1], scale=rstd)
            expw = lnp.tile([P, NQ], BF16, tag="expw")
            nc.vector.tensor_scalar_mul(out=expw, in0=sexp, scalar1=rstd)
            expw_t[t] = expw

        # ---------------- phase 2 per batch row ------------------------------
        st2 = {}

        def p2_ctxln(b):
            st2[b] = {}
            pc = ctx_ps[b]
            stats = lnp.tile([NQ, 6], F32, tag="stats2")
            nc.vector.bn_stats(out=stats, in_=pc)
            mv = lnp.tile([NQ, 2], F32, tag="mv2")
            nc.vector.bn_aggr(out=mv, in_=stats)
            rstd = lnp.tile([NQ, 1], F32, tag="rstd2")
            v2 = lnp.tile([NQ, 1], F32, tag="v2c")
            if NEWTON_MODE == "bithack":
                nc.vector.tensor_scalar_add(out=v2, in0=mv[:, 1:2], scalar1=EPS)
            else:
                nc.vector.tensor_copy(out=v2, in_=mv[:, 1:2])
            rsqrt(rstd, v2, NQ, 1, "p2c")
            ctxln = p2.tile([NQ, H], BF16, tag="ctxln")
            if "gcln" in skip:
                nc.vector.tensor_scalar(out=ctxln, in0=pc, scalar1=mv[:, 0:1],
                                        scalar2=rstd,
                                        op0=mybir.AluOpType.subtract,
                                        op1=mybir.AluOpType.mult)
            else:
                cn = p2.tile([NQ, H], F32, tag="cn")
                nc.vector.tensor_scalar(out=cn, in0=pc, scalar1=mv[:, 0:1],
                                        scalar2=rstd,
                                        op0=mybir.AluOpType.subtract,
                                        op1=mybir.AluOpType.mult)
                cgn = p2.tile([NQ, H], F32, tag="cgn")
                nc.vector.tensor_mul(out=cgn, in0=cn, in1=deferred["cg_sb"])
                nc.vector.tensor_add(out=ctxln, in0=cgn, in1=deferred["cb_sb"])
            st2[b]["ctxln"] = ctxln

        def p2_ctxT(b):
            ctxln = st2[b]["ctxln"]
            pct = pst.tile([P, 4 * NQ], BF16, tag="ps2t")
            for j in range(4):
                nc.tensor.transpose(pct[:, j * NQ:(j + 1) * NQ],
                                    ctxln[:, j * P:(j + 1) * P], id32)
            ctxT = p2.tile([P, 4, NQ], BF16, tag="ctxT")
            nc.vector.tensor_copy(out=ctxT, in_=pct)
            gcol = p2.tile([P, 4, 1], F32, tag="gcol")
            nc.vector.tensor_copy(
                out=gcol, in_=pct.rearrange("p (c q) -> p c q", q=NQ)[:, :, 0:1])

            # fused^T [128, 8, 16]: chunks 0-3 = d_ctx^T, 4-7 = g_ctx^T bcast
            fusedT = p2.tile([P, 8, D], BF16, tag="fusedT")
            for c in range(4):
                nc.vector.tensor_copy(out=fusedT[:, c, :], in_=ctxT[:, c, 1:1 + D])
            for c in range(4):
                nc.vector.tensor_scalar_mul(out=fusedT[:, 4 + c, :], in0=ones_col,
                                            scalar1=gcol[:, c, :])
            st2[b]["fusedT"] = fusedT

        def p2_ffn(b):
            fusedT = st2[b]["fusedT"]
            ph1 = pffn.tile([NQ, H], F32, tag="ps_ffn")
            for kc in range(8):
                nc.tensor.matmul(ph1[:D, :], lhsT=fusedT[:, kc, :],
                                 rhs=deferred["w1_sb"][:, kc, :], start=(kc == 0),
                                 stop=(kc == 7 and "bf1" in skip))
            if "bf1" not in skip:
                nc.tensor.matmul(ph1[:D, :], lhsT=ones_row[:, :D],
                                 rhs=deferred["bf1_sb"], start=False, stop=True)
            h1 = p2.tile([NQ, H], F32, tag="h1")
            nc.scalar.activation(out=h1[:D, :], in_=ph1[:D, :],
                                 func=mybir.ActivationFunctionType.Gelu)

            # fln LN
            stats = lnp.tile([NQ, 6], F32, tag="stats2")
            nc.vector.bn_stats(out=stats[:D, :], in_=h1[:D, :])
            mv = lnp.tile([NQ, 2], F32, tag="mv2")
            nc.vector.bn_aggr(out=mv[:D, :], in_=stats[:D, :])
            rstd = lnp.tile([NQ, 1], F32, tag="rstd2")
            v2 = lnp.tile([NQ, 1], F32, tag="v2f")
            if NEWTON_MODE == "bithack":
                nc.vector.tensor_scalar_add(out=v2[:D, :], in0=mv[:D, 1:2],
                                            scalar1=EPS)
            else:
                nc.vector.tensor_copy(out=v2[:D, :], in_=mv[:D, 1:2])
            rsqrt(rstd[:D, :], v2[:D, :], D, 1, "p2f")
            h1ln = p2.tile([NQ, H], BF16, tag="h1ln")
            if "fln" in skip:
                nc.vector.tensor_scalar(out=h1ln[:D, :], in0=h1[:D, :],
                                        scalar1=mv[:D, 0:1], scalar2=rstd[:D, :],
                                        op0=mybir.AluOpType.subtract,
                                        op1=mybir.AluOpType.mult)
            else:
                h1n = p2.tile([NQ, H], F32, tag="h1n")
                nc.vector.tensor_scalar(out=h1n[:D, :], in0=h1[:D, :],
                                        scalar1=mv[:D, 0:1], scalar2=rstd[:D, :],
                                        op0=mybir.AluOpType.subtract,
                                        op1=mybir.AluOpType.mult)
                h1g = p2.tile([NQ, H], F32, tag="h1g")
                nc.vector.tensor_mul(out=h1g[:D, :], in0=h1n[:D, :],
                                     in1=deferred["fg_sb"][:D, :])
                nc.vector.tensor_add(out=h1ln[:D, :], in0=h1g[:D, :],
                                     in1=deferred["fb_sb"][:D, :])
            st2[b]["h1ln"] = h1ln

        def p2_ffn2(b):
            h1ln = st2[b]["h1ln"]
            ph1t = pst.tile([P, 4 * NQ], BF16, tag="ps2t")
            for j in range(4):
                nc.tensor.transpose(ph1t[:, j * D:(j + 1) * D],
                                    h1ln[:D, j * P:(j + 1) * P], id32[:D, :D])
            h1T = p2.tile([P, 4, D], BF16, tag="h1T")
            nc.vector.tensor_copy(out=h1T, in_=ph1t[:, :4 * D])

            ph2 = pffn.tile([NQ, H], F32, tag="ps_ffn")
            for kc in range(4):
                nc.tensor.matmul(ph2[:D, :], lhsT=h1T[:, kc, :],
                                 rhs=deferred["w2_sb"][:, kc, :], start=(kc == 0),
                                 stop=(kc == 3 and "bf2" in skip))
            if "bf2" not in skip:
                nc.tensor.matmul(ph2[:D, :], lhsT=ones_row[:, :D],
                                 rhs=deferred["bf2_sb"], start=False, stop=True)
            h2 = p2.tile([NQ, H], F32, tag="h2")
            nc.scalar.activation(out=h2[:D, :], in_=ph2[:D, :],
                                 func=mybir.ActivationFunctionType.Gelu)

            # logits = h2 . wo + b_out_s   (wo has 1/temperature folded in)
            prod = p2.tile([NQ, H], F32, tag="prod")
            nc.vector.tensor_mul(out=prod[:D, :], in0=h2[:D, :],
                                 in1=deferred["wo_sb"][:D, :])
            lsum = lnp.tile([NQ, 1], F32, tag="lsum")
            nc.vector.reduce_sum(out=lsum[:D, :], in_=prod[:D, :],
                                 axis=mybir.AxisListType.X)
            nc.vector.tensor_scalar_add(out=logit_sb[:D, b:b + 1],
                                        in0=lsum[:D, :], scalar1=float(b_out_s))

        # driver: pipeline tiles; spread batch b's phase 2 across the next
        # batch's projection stream so PE never starves.
        p2_stages = [p2_ctxln, p2_ctxT, p2_ffn, p2_ffn2]
        stage = [0] * B_LOCAL
        _outT = out.ap().rearrange("b d -> d b")

        def advance(b):
            p2_stages[stage[b]](b)
            stage[b] += 1
            if stage[b] == len(p2_stages):
                nc.sync.dma_start(out=_outT[:, b:b + 1],
                                  in_=logit_sb[:D, b:b + 1])

        for t in range(TT):
            if t + PF < TT:
                load(t + PF)
            project(t)
            if t == 0:
                load_p2_consts()
            done = t  # tiles fully projected up to here
            for b in range(B_LOCAL - 1):
                if done >= (b + 1) * NT + stage[b] and stage[b] < len(p2_stages):
                    advance(b)
        ctx_mm(TT - 1)
        for b in range(B_LOCAL):
            while stage[b] < len(p2_stages):
                advance(b)


def _np(x):
    return np.asarray(x)


LAST_RESULT = None


def kernel(**inputs):
    from concourse.bass_utils import run_bass_kernel_spmd

    token_states = _np(inputs["token_states"]).astype(np.float32)
    mask = _np(inputs["attention_mask"])
    W_proj = _np(inputs["W_proj"]).astype(np.float64)
    b_proj = _np(inputs["b_proj"]).astype(np.float64)
    tln_g = _np(inputs["tln_g"]).astype(np.float64)
    tln_b = _np(inputs["tln_b"]).astype(np.float64)
    gln_g = _np(inputs["gln_g"]).astype(np.float32)
    gln_b = _np(inputs["gln_b"]).astype(np.float32)
    cln_g = _np(inputs["cln_g"]).astype(np.float32)
    cln_b = _np(inputs["cln_b"]).astype(np.float32)
    fln_g = _np(inputs["fln_g"]).astype(np.float32)
    fln_b = _np(inputs["fln_b"]).astype(np.float32)
    domain_queries = _np(inputs["domain_queries"]).astype(np.float64)
    global_query = _np(inputs["global_query"]).astype(np.float64)
    W_ff1 = _np(inputs["W_ff1"]).astype(np.float32)
    b_ff1 = _np(inputs["b_ff1"]).astype(np.float32)
    W_ff2 = _np(inputs["W_ff2"]).astype(np.float32)
    b_ff2 = _np(inputs["b_ff2"]).astype(np.float32)
    W_out = _np(inputs["W_out"]).astype(np.float32)
    b_out = _np(inputs["b_out"]).astype(np.float32)
    log_temperature = _np(inputs["log_temperature"]).astype(np.float32)

    Bq, Sq = mask.shape
    assert (Bq, Sq) == (B, S) and token_states.shape == (B, S, TOKD)

    # v2 fast path requires the token LN to be a pure normalization and a
    # zero projection bias (holds for this problem's inputs); the scores
    # fold and the un-normalized context trick rely on it.
    assert np.all(tln_b == 0) and np.all(b_proj == 0) \
        and np.all(tln_g == tln_g[0]), \
        "kernel v2 requires tln_b == 0, b_proj == 0, constant tln_g"

    # ---- host preprocessing ----
    counts = mask.astype(bool).sum(axis=1)
    S_c = int(max(128, -(-int(counts.max()) // P) * P))
    NT = S_c // P
    TT = B_LOCAL * NT

    bf16 = ml_dtypes.bfloat16
    ts_np = mybir.dt.np(FP8DT) if FP8 else bf16

    ts_c = np.zeros((B, S_c, TOKD), ts_np)
    padbias = np.full((B, S_c), -1e9, np.float32)
    for b in range(B):
        idx = np.flatnonzero(mask[b])
        n = len(idx)
        ts_c[b, :n] = token_states[b, idx]
        padbias[b, :n] = 0.0

    temp = float(np.clip(np.exp(log_temperature[0]), 0.3, 3.0))
    inv_t = 1.0 / temp
    wo_host = (W_out[:, 0] * inv_t).astype(np.float32)
    b_out_s = float(b_out[0] * inv_t)

    # scores fold: WM = W @ (g*q) - (W @ 1_H) * (sum(g*q)/H)
    q_all = np.concatenate([global_query[None, :], domain_queries], axis=0)  # [17,H]
    gq = (tln_g[None, :] * q_all).T                      # [H, 17]
    gq_pad = np.zeros((H, NQ), np.float64)
    gq_pad[:, :17] = gq
    sgq = gq_pad.sum(axis=0)                             # [32]
    Wgq = W_proj @ gq_pad                                # [4096, 32]
    Wrow = (W_proj @ np.ones((H, 1))) / H                # [4096, 1]
    WM = Wgq - Wrow * sgq[None, :]                       # [4096, 32]

    W_ext = np.zeros((TOKD, NS), np.float64)
    W_ext[:, :H] = W_proj
    W_ext[:, H:] = WM
    W_ext *= WSCALE
    if FP8:
        # DoubleRow rhs layout [p, pair, j, n]
        wx_host = np.ascontiguousarray(
            W_ext.reshape(KC // 2, 2, P, NS).transpose(2, 0, 1, 3)
        ).astype(ts_np).reshape(P, KC * NS)
    else:
        wx_host = np.ascontiguousarray(
            W_ext.reshape(KC, P, NS).transpose(1, 0, 2)
        ).astype(ts_np).reshape(P, KC * NS)

    cg_host = np.ones((NQ, H), np.float32)
    cb_host = np.zeros((NQ, H), np.float32)
    cg_host[0] = gln_g
    cb_host[0] = gln_b
    cg_host[1:17] = cln_g
    cb_host[1:17] = cln_b

    skip = set()
    if np.all(cg_host == 1) and np.all(cb_host == 0):
        skip.add("gcln")
    if np.all(fln_g == 1) and np.all(fln_b == 0):
        skip.add("fln")
    if np.all(b_ff1 == 0):
        skip.add("bf1")
    if np.all(b_ff2 == 0):
        skip.add("bf2")

    nc = build_nc(S_c, b_out_s, frozenset(skip))

    shared = dict(
        wx=wx_host,
        cg=cg_host, cb=cb_host,
        fg=fln_g[None, :], fb=fln_b[None, :],
        w1=W_ff1.astype(bf16), bf1=b_ff1[None, :].astype(bf16),
        w2=W_ff2.astype(bf16), bf2=b_ff2[None, :].astype(bf16),
        wo=wo_host[None, :],
    )

    in_maps = []
    for c in range(N_CORES):
        m = dict(shared)
        bs = slice(c * B_LOCAL, (c + 1) * B_LOCAL)
        # [B_LOCAL*S_c, TOKD] -> [p, t, c, tok] pre-transposed bf16
        tsl = ts_c[bs].reshape(TT, P, KC, P)
        m["tst"] = np.ascontiguousarray(
            tsl.transpose(3, 0, 2, 1)).reshape(P, TT * KC * P)
        m["pbt"] = np.ascontiguousarray(
            padbias[bs].reshape(TT, P).T)
        in_maps.append(m)

    trace = os.environ.get("KERNEL_TRACE", "0") == "1"
    kw = {}
    if trace:
        kw = dict(trace=True, tmpdir=os.environ.get("KERNEL_TRACE_DIR") or None)
    res = run_bass_kernel_spmd(nc, in_maps, core_ids=list(range(N_CORES)), **kw)
    global LAST_RESULT
    LAST_RESULT = res
    outs = [res.results[c]["out"] for c in range(N_CORES)]
    return np.concatenate(outs, axis=0).astype(np.float32)


if __name__ == "__main__":
    pass


# revision 24
# speedup vs baseline: 1.0036x; 1.0036x over previous
"""Trainium2 Bass kernel for nn_MetaRouter (dense_transformer).

Contract: kernel(**inputs) takes FULL unsharded inputs (as produced by
reference.setup_inputs()) and returns the FULL [B, D] logits, matching
reference.reference(**inputs).

Strategy (v2):
  - Data-parallel over batch: B=16 split as 2 batches per core x 8 cores.
    All parameters replicated. No collectives.
  - Host side: tokens with attention_mask==0 receive softmax weight exactly
    0 for every query, so each batch row is compacted to its unmasked
    tokens (padded to a multiple of 128; pad slots get a -1e9 score bias).
    The compacted token stream is cast to bf16 AND pre-transposed on the
    host into [128 feat-partition, tile, chunk, token] layout so the device
    runs ZERO input transposes (v1 spent ~50us of PE time on them).
  - Scores are folded into the projection matmul: for LayerNorm'd x,
        score[t,q] = rstd[t] * (ts[t,:] @ WM[:,q])  + pb[t]
    with WM = W_proj@(g*q) - (W_proj@1_H) * (sum(g*q)/H)  precomputed on
    the host (the mean-correction term is exact).  So each k-chunk issues
    one N=512 matmul (projection) plus one N=32 matmul (scores) sharing
    the same stationary tile.
  - Softmax denominator is never computed: LayerNorm(ctx) is invariant
    under per-row positive scaling and constant shifts, so the context
    matmul consumes unnormalized weights e' = exp(score)*rstd and
    UN-centered, UN-scaled projections x_raw:
        ctx_raw = sum_t e'[q,t] * px[t,:]  =  S*(ctx_true + kappa*1)
    which LayerNorms to exactly LN(ctx_true).
  - rstd = 1/sqrt(var) computed on the vector engine with Newton
    iterations (no ACT Sqrt -> no activation-table thrashing between Exp
    and Sqrt; ACT only loads Exp + Gelu tables).
  - FFN tail identical in structure to v1 (PE transposes of the tiny
    [32,512] context, two gelu matmul layers, temperature folded on host).
"""

import os

import numpy as np
import ml_dtypes

import concourse.bass as bass
import concourse.bacc as bacc
import concourse.tile as tile
from concourse import mybir
from concourse.masks import make_identity

P = 128
H = 512
TOKD = 4096
KC = TOKD // P  # 32 k-chunks of the projection contraction
NQ = 32         # 17 queries (1 global + 16 domains) padded to 32
D = 16
B = 16
S = 2048
N_CORES = 8
B_LOCAL = B // N_CORES
EPS = 1e-5
F32 = mybir.dt.float32
BF16 = mybir.dt.bfloat16
U32 = mybir.dt.uint32
NS = H + NQ  # projection + score columns

NEWTON_MODE = os.environ.get("KERNEL_NEWTON", "bithack")  # bithack | act
# fp8e4m3 DoubleRow was tried and runs ~1.3x faster on the PE, but its
# ~4-5% per-product quantization error propagates to ~3.3e-2 relative
# error on the logits (gate: 2e-2) -- attention averaging does not shrink
# it. Keep bf16.
FP8 = os.environ.get("KERNEL_FP8", "0") == "1"
FP8DT = mybir.dt.float8e4
TS_DT = FP8DT if FP8 else BF16
# LayerNorm makes the projection scale-invariant, so the weights can be
# pre-scaled into fp8e4m3's sweet spot (w ~ N(0, 0.02) -> ~N(0, 1.3)).
WSCALE = 64.0 if FP8 else 1.0


def build_nc(S_c: int, b_out_s: float, skip=frozenset()):
    """Build the per-core Bass program for padded/compacted seq length S_c."""
    assert S_c % P == 0
    NT = S_c // P          # token tiles per batch row
    TT = B_LOCAL * NT      # token tiles per core

    nc = bacc.Bacc("TRN2", target_bir_lowering=False, num_swdge_queues=2)

    # host-pretransposed token stream: [p, t, c, tok] = ts[t*128+tok, c*128+p]
    tst = nc.declare_dram_parameter("tst", [P, TT * KC * P], TS_DT, isOutput=False)
    # host layout [p, c, n] = W_ext[c*128+p, n];  W_ext = [W_proj | WM]
    # (fp8: [p, pair, j, n] = W_ext[(2*pair+j)*128+p, n] for DoubleRow rhs)
    wx = nc.declare_dram_parameter("wx", [P, KC * NS], TS_DT, isOutput=False)
    # pad score bias, [p, t] = 0.0 for live tokens / -1e9 for pad slots
    pbt = nc.declare_dram_parameter("pbt", [P, TT], F32, isOutput=False)
    cg = nc.declare_dram_parameter("cg", [NQ, H], F32, isOutput=False)
    cb = nc.declare_dram_parameter("cb", [NQ, H], F32, isOutput=False)
    fg = nc.declare_dram_parameter("fg", [1, H], F32, isOutput=False)
    fb = nc.declare_dram_parameter("fb", [1, H], F32, isOutput=False)
    w1 = nc.declare_dram_parameter("w1", [2 * H, H], BF16, isOutput=False)
    bf1 = nc.declare_dram_parameter("bf1", [1, H], BF16, isOutput=False)
    w2 = nc.declare_dram_parameter("w2", [H, H], BF16, isOutput=False)
    bf2 = nc.declare_dram_parameter("bf2", [1, H], BF16, isOutput=False)
    wo = nc.declare_dram_parameter("wo", [1, H], F32, isOutput=False)
    out = nc.declare_dram_parameter("out", [B_LOCAL, D], F32, isOutput=True)

    with tile.TileContext(nc) as tc:
        _emit(tc, nc, S_c, NT, TT, b_out_s, skip,
              tst=tst, wx=wx, pbt=pbt, cg=cg, cb=cb, fg=fg, fb=fb,
              w1=w1, bf1=bf1, w2=w2, bf2=bf2, wo=wo, out=out)
    nc.compile()
    return nc


def _emit(tc, nc, S_c, NT, TT, b_out_s, skip, *, tst, wx, pbt, cg, cb,
          fg, fb, w1, bf1, w2, bf2, wo, out):
    from contextlib import ExitStack
    ctx = ExitStack()
    with ctx:
        const = ctx.enter_context(tc.tile_pool(name="const", bufs=1))
        tsp = ctx.enter_context(tc.tile_pool(name="tsp", bufs=4))
        xp = ctx.enter_context(tc.tile_pool(name="xp", bufs=1))
        lnp = ctx.enter_context(tc.tile_pool(name="lnp", bufs=4))
        p2 = ctx.enter_context(tc.tile_pool(name="p2", bufs=1))
        psx = ctx.enter_context(tc.tile_pool(name="psx", bufs=2, space="PSUM"))
        pssc = ctx.enter_context(tc.tile_pool(name="pssc", bufs=2, space="PSUM"))
        pctx = ctx.enter_context(tc.tile_pool(name="pctx", bufs=2, space="PSUM"))
        pst = ctx.enter_context(tc.tile_pool(name="pst", bufs=1, space="PSUM"))
        pffn = ctx.enter_context(tc.tile_pool(name="pffn", bufs=1, space="PSUM"))

        # ---- weight + const loads; k-interleaved across the two HWDGE
        # rings so chunk k arrives roughly in consumption order ----
        if FP8:
            w_sb = const.tile([P, KC // 2, 2, NS], TS_DT)
            _wx = wx.ap().rearrange("p (c j n) -> p c j n", c=KC // 2, j=2)
            for _q in range(KC // 2):
                eng = nc.sync if _q % 2 == 0 else nc.scalar
                eng.dma_start(out=w_sb[:, _q:_q + 1, :, :],
                              in_=_wx[:, _q:_q + 1, :, :])
        else:
            w_sb = const.tile([P, KC, NS], TS_DT)
            _wx = wx.ap().rearrange("p (c n) -> p c n", c=KC)
            for _q in range(KC // 2):
                _qs = slice(_q * 2, _q * 2 + 2)
                eng = nc.sync if _q % 2 == 0 else nc.scalar
                eng.dma_start(out=w_sb[:, _qs, :], in_=_wx[:, _qs, :])

        pbt_sb = const.tile([P, TT], F32)
        nc.sync.dma_start(out=pbt_sb, in_=pbt.ap())

        ts_tiles = [None] * TT
        _tst = tst.ap().rearrange("p (t x) -> p t x", t=TT)

        load_engines = [nc.gpsimd, nc.sync, nc.gpsimd, nc.scalar]

        def load(t):
            tile_ = tsp.tile([P, KC, P], TS_DT, tag="ts")
            src = _tst[:, t, :].rearrange("p (c x) -> p c x", c=KC)
            if t < 2:
                # startup: split across the SWDGE queues for parallelism
                for s in range(4):
                    cs = slice(s * (KC // 4), (s + 1) * (KC // 4))
                    nc.gpsimd.dma_start(out=tile_[:, cs, :], in_=src[:, cs, :])
            else:
                eng = load_engines[t % len(load_engines)]
                eng.dma_start(out=tile_, in_=src)
            ts_tiles[t] = tile_

        PF = 5
        for _t in range(min(PF, TT)):
            load(_t)

        deferred = {}

        def bcast(dram, parts, dt=F32):
            t = const.tile([parts, H], dt, tag=f"c_{dram.name}")
            a = dram.ap()
            nc.scalar.dma_start(
                out=t, in_=bass.AP(tensor=a.tensor, offset=a.offset,
                                   ap=[[0, parts]] + list(a.ap[1:])))
            return t

        def load_p2_consts():
            w1_sb = const.tile([P, 8, H], BF16)
            nc.scalar.dma_start(out=w1_sb,
                                in_=w1.ap().rearrange("(c p) h -> p c h", p=P))
            w2_sb = const.tile([P, 4, H], BF16)
            nc.scalar.dma_start(out=w2_sb,
                                in_=w2.ap().rearrange("(c p) h -> p c h", p=P))
            fg_sb = bcast(fg, NQ)
            fb_sb = bcast(fb, NQ)
            wo_sb = bcast(wo, NQ)
            cg_sb = const.tile([NQ, H], F32)
            nc.scalar.dma_start(out=cg_sb, in_=cg.ap())
            cb_sb = const.tile([NQ, H], F32)
            nc.scalar.dma_start(out=cb_sb, in_=cb.ap())
            bf1_sb = const.tile([1, H], BF16)
            nc.scalar.dma_start(out=bf1_sb, in_=bf1.ap())
            bf2_sb = const.tile([1, H], BF16)
            nc.scalar.dma_start(out=bf2_sb, in_=bf2.ap())
            deferred.update(w1_sb=w1_sb, w2_sb=w2_sb, fg_sb=fg_sb,
                            fb_sb=fb_sb, wo_sb=wo_sb, cg_sb=cg_sb, cb_sb=cb_sb,
                            bf1_sb=bf1_sb, bf2_sb=bf2_sb)

        ones_row = const.tile([1, P], BF16)
        nc.vector.memset(ones_row, 1.0)
        ones_col = const.tile([P, D], BF16)
        nc.vector.memset(ones_col, 1.0)
        id32 = const.tile([NQ, NQ], BF16)
        make_identity(nc, id32)
        magic_u = const.tile([P, max(TT, NQ)], U32)
        nc.vector.memset(magic_u, 0x5f3759df)
        eps_sb = const.tile([P, 1], F32)
        nc.vector.memset(eps_sb, EPS)

        # persistent activations
        x_raw = xp.tile([P, TT, H], BF16)       # un-normalized projections
        logit_sb = xp.tile([NQ, B_LOCAL], F32)

        def newton_rsqrt(out_ap, v_ap, p, n, tag):
            """out = 1/sqrt(v) elementwise, on DVE only (no ACT table).

            Bit-hack seed + 2 Newton steps: ~4e-6 rel error in fp32.
            """
            sh = lnp.tile([p, n], U32, tag=f"nw_sh_{tag}")
            nc.vector.tensor_scalar(out=sh, in0=v_ap.bitcast(U32),
                                    scalar1=1, scalar2=None,
                                    op0=mybir.AluOpType.logical_shift_right)
            y = lnp.tile([p, n], F32, tag=f"nw_y_{tag}")
            nc.vector.tensor_sub(out=y.bitcast(U32), in0=magic_u[:p, :n],
                                 in1=sh)
            t_ = lnp.tile([p, n], F32, tag=f"nw_t_{tag}")
            for it in range(2):
                nc.vector.tensor_mul(out=t_, in0=y, in1=y)
                nc.vector.scalar_tensor_tensor(
                    out=t_, in0=t_, scalar=-0.5, in1=v_ap,
                    op0=mybir.AluOpType.mult, op1=mybir.AluOpType.mult)
                nc.vector.scalar_tensor_tensor(
                    out=(y if it == 0 else out_ap), in0=t_, scalar=1.5, in1=y,
                    op0=mybir.AluOpType.add, op1=mybir.AluOpType.mult)

        def act_rsqrt(out_ap, v_ap, p, n, tag):
            s = lnp.tile([p, n], F32, tag=f"as_{tag}")
            nc.scalar.activation(out=s, in_=v_ap,
                                 func=mybir.ActivationFunctionType.Sqrt,
                                 bias=eps_sb[:p], scale=1.0)
            nc.vector.reciprocal(out=out_ap, in_=s)

        rsqrt = newton_rsqrt if NEWTON_MODE == "bithack" else act_rsqrt

        # ---------------- phase 1: project + scores + ctx per token tile ----
        ctx_ps = [None] * B_LOCAL
        expw_t = [None] * TT

        def ctx_mm(t):
            # emitted one tile late so expw(t) is ready when PE reaches it
            b, i = divmod(t, NT)
            if i == 0:
                pc = pctx.tile([NQ, H], F32, tag="pc")
                ctx_ps[b] = pc
            nc.tensor.matmul(ctx_ps[b], lhsT=expw_t[t], rhs=x_raw[:, t, :],
                             start=(i == 0), stop=(i == NT - 1))
            expw_t[t] = None

        def project(t):
            ts_sb = ts_tiles[t]
            px = psx.tile([P, H], F32, tag="px")
            sc = pssc.tile([P, NQ], F32, tag="sc")
            if FP8:
                DR = mybir.MatmulPerfMode.DoubleRow
                sc_mode = os.environ.get("KERNEL_SC_MODE", "dr")
                for p in range(KC // 2):
                    lt = ts_sb[:, 2 * p:2 * p + 2, :]
                    nc.tensor.matmul(px, lhsT=lt, rhs=w_sb[:, p, :, :H],
                                     start=(p == 0), stop=(p == KC // 2 - 1),
                                     perf_mode=DR)
                    if sc_mode == "dr":
                        nc.tensor.matmul(sc, lhsT=lt, rhs=w_sb[:, p, :, H:],
                                         start=(p == 0),
                                         stop=(p == KC // 2 - 1),
                                         perf_mode=DR)
                if sc_mode == "single":
                    for k in range(KC):
                        nc.tensor.matmul(sc, lhsT=ts_sb[:, k, :],
                                         rhs=w_sb[:, k // 2, k % 2, H:],
                                         start=(k == 0), stop=(k == KC - 1))
                elif sc_mode == "off":
                    nc.vector.memset(sc, 0.0)
            else:
                for k in range(KC):
                    nc.tensor.matmul(px, lhsT=ts_sb[:, k, :],
                                     rhs=w_sb[:, k, :H],
                                     start=(k == 0), stop=(k == KC - 1))
                    nc.tensor.matmul(sc, lhsT=ts_sb[:, k, :],
                                     rhs=w_sb[:, k, H:],
                                     start=(k == 0), stop=(k == KC - 1))
            ts_tiles[t] = None
            if t > 0:
                ctx_mm(t - 1)
            stats = lnp.tile([P, 6], F32, tag="stats")
            nc.vector.bn_stats(out=stats, in_=px)
            mv = lnp.tile([P, 2], F32, tag="mv")
            nc.vector.bn_aggr(out=mv, in_=stats)
            nc.scalar.copy(out=x_raw[:, t, :], in_=px)
            # rstd for this tile's tokens (per-token over H -> tile-local)
            v = lnp.tile([P, 1], F32, tag="ptv")
            nc.vector.tensor_scalar_add(out=v, in0=mv[:, 1:2], scalar1=EPS)
            rstd = lnp.tile([P, 1], F32, tag="ptr")
            rsqrt(rstd, v, P, 1, "p1")
            # unnormalized softmax numerator, rstd folded for the ctx trick
            sexp = lnp.tile([P, NQ], F32, tag="sexp")
            nc.scalar.activation(out=sexp, in_=sc,
                                 func=mybir.ActivationFunctionType.Exp,
                                 bias=pbt_sb[:, t:t + 1], scale=rstd)
            expw = lnp.tile([P, NQ], BF16, tag="expw")
            nc.vector.tensor_scalar_mul(out=expw, in0=sexp, scalar1=rstd)
            expw_t[t] = expw

        # ---------------- phase 2 per batch row ------------------------------
        st2 = {}

        def p2_ctxln(b):
            st2[b] = {}
            pc = ctx_ps[b]
            stats = lnp.tile([NQ, 6], F32, tag="stats2")
            nc.vector.bn_stats(out=stats, in_=pc)
            mv = lnp.tile([NQ, 2], F32, tag="mv2")
            nc.vector.bn_aggr(out=mv, in_=stats)
            rstd = lnp.tile([NQ, 1], F32, tag="rstd2")
            v2 = lnp.tile([NQ, 1], F32, tag="v2c")
            if NEWTON_MODE == "bithack":
                nc.vector.tensor_scalar_add(out=v2, in0=mv[:, 1:2], scalar1=EPS)
            else:
                nc.vector.tensor_copy(out=v2, in_=mv[:, 1:2])
            rsqrt(rstd, v2, NQ, 1, "p2c")
            ctxln = p2.tile([NQ, H], BF16, tag="ctxln")
            if "gcln" in skip:
                nc.vector.tensor_scalar(out=ctxln, in0=pc, scalar1=mv[:, 0:1],
                                        scalar2=rstd,
                                        op0=mybir.AluOpType.subtract,
                                        op1=mybir.AluOpType.mult)
            else:
                cn = p2.tile([NQ, H], F32, tag="cn")
                nc.vector.tensor_scalar(out=cn, in0=pc, scalar1=mv[:, 0:1],
                                        scalar2=rstd,
                                        op0=mybir.AluOpType.subtract,
                                        op1=mybir.AluOpType.mult)
                cgn = p2.tile([NQ, H], F32, tag="cgn")
                nc.vector.tensor_mul(out=cgn, in0=cn, in1=deferred["cg_sb"])
                nc.vector.tensor_add(out=ctxln, in0=cgn, in1=deferred["cb_sb"])
            st2[b]["ctxln"] = ctxln

        def p2_ctxT(b):
            ctxln = st2[b]["ctxln"]
            pct = pst.tile([P, 4 * NQ], BF16, tag="ps2t")
            for j in range(4):
                nc.tensor.transpose(pct[:, j * NQ:(j + 1) * NQ],
                                    ctxln[:, j * P:(j + 1) * P], id32)
            ctxT = p2.tile([P, 4, NQ], BF16, tag="ctxT")
            nc.vector.tensor_copy(out=ctxT, in_=pct)
            gcol = p2.tile([P, 4, 1], F32, tag="gcol")
            nc.vector.tensor_copy(
                out=gcol, in_=pct.rearrange("p (c q) -> p c q", q=NQ)[:, :, 0:1])

            # fused^T [128, 8, 16]: chunks 0-3 = d_ctx^T, 4-7 = g_ctx^T bcast
            fusedT = p2.tile([P, 8, D], BF16, tag="fusedT")
            for c in range(4):
                nc.vector.tensor_copy(out=fusedT[:, c, :], in_=ctxT[:, c, 1:1 + D])
            for c in range(4):
                nc.vector.tensor_scalar_mul(out=fusedT[:, 4 + c, :], in0=ones_col,
                                            scalar1=gcol[:, c, :])
            st2[b]["fusedT"] = fusedT

        def p2_ffn(b):
            fusedT = st2[b]["fusedT"]
            ph1 = pffn.tile([NQ, H], F32, tag="ps_ffn")
            for kc in range(8):
                nc.tensor.matmul(ph1[:D, :], lhsT=fusedT[:, kc, :],
                                 rhs=deferred["w1_sb"][:, kc, :], start=(kc == 0),
                                 stop=(kc == 7 and "bf1" in skip))
            if "bf1" not in skip:
                nc.tensor.matmul(ph1[:D, :], lhsT=ones_row[:, :D],
                                 rhs=deferred["bf1_sb"], start=False, stop=True)
            h1 = p2.tile([NQ, H], F32, tag="h1")
            nc.scalar.activation(out=h1[:D, :], in_=ph1[:D, :],
                                 func=mybir.ActivationFunctionType.Gelu)

            # fln LN
            stats = lnp.tile([NQ, 6], F32, tag="stats2")
            nc.vector.bn_stats(out=stats[:D, :], in_=h1[:D, :])
            mv = lnp.tile([NQ, 2], F32, tag="mv2")
            nc.vector.bn_aggr(out=mv[:D, :], in_=stats[:D, :])
            rstd = lnp.tile([NQ, 1], F32, tag="rstd2")
            v2 = lnp.tile([NQ, 1], F32, tag="v2f")
            if NEWTON_MODE == "bithack":
                nc.vector.tensor_scalar_add(out=v2[:D, :], in0=mv[:D, 1:2],
                                            scalar1=EPS)
            else:
                nc.vector.tensor_copy(out=v2[:D, :], in_=mv[:D, 1:2])
            rsqrt(rstd[:D, :], v2[:D, :], D, 1, "p2f")
            h1ln = p2.tile([NQ, H], BF16, tag="h1ln")
            if "fln" in skip:
                nc.vector.tensor_scalar(out=h1ln[:D, :], in0=h1[:D, :],
                                        scalar1=mv[:D, 0:1], scalar2=rstd[:D, :],
                                        op0=mybir.AluOpType.subtract,
                                        op1=mybir.AluOpType.mult)
            else:
                h1n = p2.tile([NQ, H], F32, tag="h1n")
                nc.vector.tensor_scalar(out=h1n[:D, :], in0=h1[:D, :],
                                        scalar1=mv[:D, 0:1], scalar2=rstd[:D, :],
                                        op0=mybir.AluOpType.subtract,
                                        op1=mybir.AluOpType.mult)
                h1g = p2.tile([NQ, H], F32, tag="h1g")
                nc.vector.tensor_mul(out=h1g[:D, :], in0=h1n[:D, :],
                                     in1=deferred["fg_sb"][:D, :])
                nc.vector.tensor_add(out=h1ln[:D, :], in0=h1g[:D, :],
                                     in1=deferred["fb_sb"][:D, :])
            st2[b]["h1ln"] = h1ln

        def p2_ffn2(b):
            h1ln = st2[b]["h1ln"]
            ph1t = pst.tile([P, 4 * NQ], BF16, tag="ps2t")
            for j in range(4):
                nc.tensor.transpose(ph1t[:, j * D:(j + 1) * D],
                                    h1ln[:D, j * P:(j + 1) * P], id32[:D, :D])
            h1T = p2.tile([P, 4, D], BF16, tag="h1T")
            nc.vector.tensor_copy(out=h1T, in_=ph1t[:, :4 * D])

            ph2 = pffn.tile([NQ, H], F32, tag="ps_ffn")
            for kc in range(4):
                nc.tensor.matmul(ph2[:D, :], lhsT=h1T[:, kc, :],
                                 rhs=deferred["w2_sb"][:, kc, :], start=(kc == 0),
                                 stop=(kc == 3 and "bf2" in skip))
            if "bf2" not in skip:
                nc.tensor.matmul(ph2[:D, :], lhsT=ones_row[:, :D],
                                 rhs=deferred["bf2_sb"], start=False, stop=True)
            h2 = p2.tile([NQ, H], F32, tag="h2")
            nc.scalar.activation(out=h2[:D, :], in_=ph2[:D, :],
                                 func=mybir.ActivationFunctionType.Gelu)

            # logits = h2 . wo + b_out_s   (wo has 1/temperature folded in)
            prod = p2.tile([NQ, H], F32, tag="prod")
            nc.vector.tensor_mul(out=prod[:D, :], in0=h2[:D, :],
                                 in1=deferred["wo_sb"][:D, :])
            lsum = lnp.tile([NQ, 1], F32, tag="lsum")
            nc.vector.reduce_sum(out=lsum[:D, :], in_=prod[:D, :],
                                 axis=mybir.AxisListType.X)
            nc.vector.tensor_scalar_add(out=logit_sb[:D, b:b + 1],
                                        in0=lsum[:D, :], scalar1=float(b_out_s))

        # driver: pipeline tiles; spread batch b's phase 2 across the next
        # batch's projection stream so PE never starves.
        p2_stages = [p2_ctxln, p2_ctxT, p2_ffn, p2_ffn2]
        stage = [0] * B_LOCAL
        _outT = out.ap().rearrange("b d -> d b")

        def advance(b):
            p2_stages[stage[b]](b)
            stage[b] += 1
            if stage[b] == len(p2_stages):
                nc.sync.dma_start(out=_outT[:, b:b + 1],
                                  in_=logit_sb[:D, b:b + 1])

        for t in range(TT):
            if t + PF < TT:
                load(t + PF)
            project(t)
            if t == 0:
                load_p2_consts()
            done = t  # tiles fully projected up to here
            for b in range(B_LOCAL - 1):
                if done >= (b + 1) * NT + stage[b] and stage[b] < len(p2_stages):
                    advance(b)
        ctx_mm(TT - 1)
        for b in range(B_LOCAL):
            while stage[b] < len(p2_stages):
                advance(b)


def _np(x):
    return np.asarray(x)


LAST_RESULT = None


def kernel(**inputs):
    from concourse.bass_utils import run_bass_kernel_spmd

    token_states = _np(inputs["token_states"]).astype(np.float32)
    mask = _np(inputs["attention_mask"])
    W_proj = _np(inputs["W_proj"]).astype(np.float64)
    b_proj = _np(inputs["b_proj"]).astype(np.float64)
    tln_g = _np(inputs["tln_g"]).astype(np.float64)
    tln_b = _np(inputs["tln_b"]).astype(np.float64)
    gln_g = _np(inputs["gln_g"]).astype(np.float32)
    gln_b = _np(inputs["gln_b"]).astype(np.float32)
    cln_g = _np(inputs["cln_g"]).astype(np.float32)
    cln_b = _np(inputs["cln_b"]).astype(np.float32)
    fln_g = _np(inputs["fln_g"]).astype(np.float32)
    fln_b = _np(inputs["fln_b"]).astype(np.float32)
    domain_queries = _np(inputs["domain_queries"]).astype(np.float64)
    global_query = _np(inputs["global_query"]).astype(np.float64)
    W_ff1 = _np(inputs["W_ff1"]).astype(np.float32)
    b_ff1 = _np(inputs["b_ff1"]).astype(np.float32)
    W_ff2 = _np(inputs["W_ff2"]).astype(np.float32)
    b_ff2 = _np(inputs["b_ff2"]).astype(np.float32)
    W_out = _np(inputs["W_out"]).astype(np.float32)
    b_out = _np(inputs["b_out"]).astype(np.float32)
    log_temperature = _np(inputs["log_temperature"]).astype(np.float32)

    Bq, Sq = mask.shape
    assert (Bq, Sq) == (B, S) and token_states.shape == (B, S, TOKD)

    # v2 fast path requires the token LN to be a pure normalization and a
    # zero projection bias (holds for this problem's inputs); the scores
    # fold and the un-normalized context trick rely on it.
    assert np.all(tln_b == 0) and np.all(b_proj == 0) \
        and np.all(tln_g == tln_g[0]), \
        "kernel v2 requires tln_b == 0, b_proj == 0, constant tln_g"

    # ---- host preprocessing ----
    counts = mask.astype(bool).sum(axis=1)
    S_c = int(max(128, -(-int(counts.max()) // P) * P))
    NT = S_c // P
    TT = B_LOCAL * NT

    bf16 = ml_dtypes.bfloat16
    ts_np = mybir.dt.np(FP8DT) if FP8 else bf16

    ts_c = np.zeros((B, S_c, TOKD), ts_np)
    padbias = np.full((B, S_c), -1e9, np.float32)
    for b in range(B):
        idx = np.flatnonzero(mask[b])
        n = len(idx)
        ts_c[b, :n] = token_states[b, idx]
        padbias[b, :n] = 0.0

    temp = float(np.clip(np.exp(log_temperature[0]), 0.3, 3.0))
    inv_t = 1.0 / temp
    wo_host = (W_out[:, 0] * inv_t).astype(np.float32)
    b_out_s = float(b_out[0] * inv_t)

    # scores fold: WM = W @ (g*q) - (W @ 1_H) * (sum(g*q)/H)
    q_all = np.concatenate([global_query[None, :], domain_queries], axis=0)  # [17,H]
    gq = (tln_g[None, :] * q_all).T                      # [H, 17]
    gq_pad = np.zeros((H, NQ), np.float64)
    gq_pad[:, :17] = gq
    sgq = gq_pad.sum(axis=0)                             # [32]
    Wgq = W_proj @ gq_pad                                # [4096, 32]
    Wrow = (W_proj @ np.ones((H, 1))) / H                # [4096, 1]
    WM = Wgq - Wrow * sgq[None, :]                       # [4096, 32]

    W_ext = np.zeros((TOKD, NS), np.float64)
    W_ext[:, :H] = W_proj
    W_ext[:, H:] = WM
    W_ext *= WSCALE
    if FP8:
        # DoubleRow rhs layout [p, pair, j, n]
        wx_host = np.ascontiguousarray(
            W_ext.reshape(KC // 2, 2, P, NS).transpose(2, 0, 1, 3)
        ).astype(ts_np).reshape(P, KC * NS)
    else:
        wx_host = np.ascontiguousarray(
            W_ext.reshape(KC, P, NS).transpose(1, 0, 2)
        ).astype(ts_np).reshape(P, KC * NS)

    cg_host = np.ones((NQ, H), np.float32)
    cb_host = np.zeros((NQ, H), np.float32)
    cg_host[0] = gln_g
    cb_host[0] = gln_b
    cg_host[1:17] = cln_g
    cb_host[1:17] = cln_b

    skip = set()
    if np.all(cg_host == 1) and np.all(cb_host == 0):
        skip.add("gcln")
    if np.all(fln_g == 1) and np.all(fln_b == 0):
        skip.add("fln")
    if np.all(b_ff1 == 0):
        skip.add("bf1")
    if np.all(b_ff2 == 0):
        skip.add("bf2")

    nc = build_nc(S_c, b_out_s, frozenset(skip))

    shared = dict(
        wx=wx_host,
        cg=cg_host, cb=cb_host,
        fg=fln_g[None, :], fb=fln_b[None, :],
        w1=W_ff1.astype(bf16), bf1=b_ff1[None, :].astype(bf16),
        w2=W_ff2.astype(bf16), bf2=b_ff2[None, :].astype(bf16),
        wo=wo_host[None, :],
    )

    in_maps = []
    for c in range(N_CORES):
        m = dict(shared)
        bs = slice(c * B_LOCAL, (c + 1) * B_LOCAL)
        # [B_LOCAL*S_c, TOKD] -> [p, t, c, tok] pre-transposed bf16
        tsl = ts_c[bs].reshape(TT, P, KC, P)
        m["tst"] = np.ascontiguousarray(
            tsl.transpose(3, 0, 2, 1)).reshape(P, TT * KC * P)
        m["pbt"] = np.ascontiguousarray(
            padbias[bs].reshape(TT, P).T)
        in_maps.append(m)

    trace = os.environ.get("KERNEL_TRACE", "0") == "1"
    kw = {}
    if trace:
        kw = dict(trace=True, tmpdir=os.environ.get("KERNEL_TRACE_DIR") or None)
    res = run_bass_kernel_spmd(nc, in_maps, core_ids=list(range(N_CORES)), **kw)
    global LAST_RESULT
    LAST_RESULT = res
    outs = [res.results[c]["out"] for c in range(N_CORES)]
    return np.concatenate(outs, axis=0).astype(np.float32)


if __name__ == "__main__":
    pass


# revision 25
# speedup vs baseline: 1.1559x; 1.1518x over previous
"""Trainium2 Bass kernel for nn_MetaRouter (dense_transformer).

Contract: kernel(**inputs) takes FULL unsharded inputs (as produced by
reference.setup_inputs()) and returns the FULL [B, D] logits, matching
reference.reference(**inputs).

Strategy (v2):
  - Data-parallel over batch: B=16 split as 2 batches per core x 8 cores.
    All parameters replicated. No collectives.
  - Host side: tokens with attention_mask==0 receive softmax weight exactly
    0 for every query, so each batch row is compacted to its unmasked
    tokens (padded to a multiple of 128; pad slots get a -1e9 score bias).
    The compacted token stream is cast to bf16 AND pre-transposed on the
    host into [128 feat-partition, tile, chunk, token] layout so the device
    runs ZERO input transposes (v1 spent ~50us of PE time on them).
  - Scores are folded into the projection matmul: for LayerNorm'd x,
        score[t,q] = rstd[t] * (ts[t,:] @ WM[:,q])  + pb[t]
    with WM = W_proj@(g*q) - (W_proj@1_H) * (sum(g*q)/H)  precomputed on
    the host (the mean-correction term is exact).  So each k-chunk issues
    one N=512 matmul (projection) plus one N=32 matmul (scores) sharing
    the same stationary tile.
  - Softmax denominator is never computed: LayerNorm(ctx) is invariant
    under per-row positive scaling and constant shifts, so the context
    matmul consumes unnormalized weights e' = exp(score)*rstd and
    UN-centered, UN-scaled projections x_raw:
        ctx_raw = sum_t e'[q,t] * px[t,:]  =  S*(ctx_true + kappa*1)
    which LayerNorms to exactly LN(ctx_true).
  - rstd = 1/sqrt(var) computed on the vector engine with Newton
    iterations (no ACT Sqrt -> no activation-table thrashing between Exp
    and Sqrt; ACT only loads Exp + Gelu tables).
  - FFN tail identical in structure to v1 (PE transposes of the tiny
    [32,512] context, two gelu matmul layers, temperature folded on host).
"""

import os

import numpy as np
import ml_dtypes

import concourse.bass as bass
import concourse.bacc as bacc
import concourse.tile as tile
from concourse import mybir
from concourse.masks import make_identity

P = 128
H = 512
TOKD = 4096
KC = TOKD // P  # 32 k-chunks of the projection contraction
NQ = 32         # 17 queries (1 global + 16 domains) padded to 32
D = 16
B = 16
S = 2048
N_CORES = 8
B_LOCAL = B // N_CORES
EPS = 1e-5
F32 = mybir.dt.float32
BF16 = mybir.dt.bfloat16
U32 = mybir.dt.uint32
NS = H + NQ  # projection + score columns

NEWTON_MODE = os.environ.get("KERNEL_NEWTON", "bithack")  # bithack | act
# fp8e4m3 DoubleRow was tried and runs ~1.3x faster on the PE, but its
# ~4-5% per-product quantization error propagates to ~3.3e-2 relative
# error on the logits (gate: 2e-2) -- attention averaging does not shrink
# it. Keep bf16.
FP8 = os.environ.get("KERNEL_FP8", "0") == "1"
FP8DT = mybir.dt.float8e4
TS_DT = FP8DT if FP8 else BF16
# LayerNorm makes the projection scale-invariant, so the weights can be
# pre-scaled into fp8e4m3's sweet spot (w ~ N(0, 0.02) -> ~N(0, 1.3)).
WSCALE = 64.0 if FP8 else 1.0


def build_nc(S_c: int, b_out_s: float, skip=frozenset()):
    """Build the per-core Bass program for padded/compacted seq length S_c."""
    assert S_c % P == 0
    NT = S_c // P          # token tiles per batch row
    TT = B_LOCAL * NT      # token tiles per core

    nc = bacc.Bacc("TRN2", target_bir_lowering=False, num_swdge_queues=2)

    # host-pretransposed token stream: [p, t, c, tok] = ts[t*128+tok, c*128+p]
    tst = nc.declare_dram_parameter("tst", [P, TT * KC * P], TS_DT, isOutput=False)
    # host layout [p, c, n] = W_ext[c*128+p, n];  W_ext = [W_proj | WM]
    # (fp8: [p, pair, j, n] = W_ext[(2*pair+j)*128+p, n] for DoubleRow rhs)
    wx = nc.declare_dram_parameter("wx", [P, KC * NS], TS_DT, isOutput=False)
    # pad score bias, [p, t] = 0.0 for live tokens / -1e9 for pad slots
    pbt = nc.declare_dram_parameter("pbt", [P, TT], F32, isOutput=False)
    cg = nc.declare_dram_parameter("cg", [NQ, H], F32, isOutput=False)
    cb = nc.declare_dram_parameter("cb", [NQ, H], F32, isOutput=False)
    fg = nc.declare_dram_parameter("fg", [1, H], F32, isOutput=False)
    fb = nc.declare_dram_parameter("fb", [1, H], F32, isOutput=False)
    w1 = nc.declare_dram_parameter("w1", [2 * H, H], BF16, isOutput=False)
    bf1 = nc.declare_dram_parameter("bf1", [1, H], BF16, isOutput=False)
    w2 = nc.declare_dram_parameter("w2", [H, H], BF16, isOutput=False)
    bf2 = nc.declare_dram_parameter("bf2", [1, H], BF16, isOutput=False)
    wo = nc.declare_dram_parameter("wo", [1, H], F32, isOutput=False)
    out = nc.declare_dram_parameter("out", [B_LOCAL, D], F32, isOutput=True)

    with tile.TileContext(nc) as tc:
        _emit(tc, nc, S_c, NT, TT, b_out_s, skip,
              tst=tst, wx=wx, pbt=pbt, cg=cg, cb=cb, fg=fg, fb=fb,
              w1=w1, bf1=bf1, w2=w2, bf2=bf2, wo=wo, out=out)
    nc.compile()
    return nc


def _emit(tc, nc, S_c, NT, TT, b_out_s, skip, *, tst, wx, pbt, cg, cb,
          fg, fb, w1, bf1, w2, bf2, wo, out):
    from contextlib import ExitStack
    ctx = ExitStack()
    with ctx:
        const = ctx.enter_context(tc.tile_pool(name="const", bufs=1))
        tsp = ctx.enter_context(tc.tile_pool(name="tsp", bufs=6))
        xp = ctx.enter_context(tc.tile_pool(name="xp", bufs=1))
        lnp = ctx.enter_context(tc.tile_pool(name="lnp", bufs=4))
        p2 = ctx.enter_context(tc.tile_pool(name="p2", bufs=1))
        psx = ctx.enter_context(tc.tile_pool(name="psx", bufs=2, space="PSUM"))
        pssc = ctx.enter_context(tc.tile_pool(name="pssc", bufs=2, space="PSUM"))
        pctx = ctx.enter_context(tc.tile_pool(name="pctx", bufs=2, space="PSUM"))
        pst = ctx.enter_context(tc.tile_pool(name="pst", bufs=1, space="PSUM"))
        pffn = ctx.enter_context(tc.tile_pool(name="pffn", bufs=1, space="PSUM"))

        # ---- weight + const loads; k-interleaved across the two HWDGE
        # rings so chunk k arrives roughly in consumption order ----
        if FP8:
            w_sb = const.tile([P, KC // 2, 2, NS], TS_DT)
            _wx = wx.ap().rearrange("p (c j n) -> p c j n", c=KC // 2, j=2)
            for _q in range(KC // 2):
                eng = nc.sync if _q % 2 == 0 else nc.scalar
                eng.dma_start(out=w_sb[:, _q:_q + 1, :, :],
                              in_=_wx[:, _q:_q + 1, :, :])
        else:
            w_sb = const.tile([P, KC, NS], TS_DT)
            _wx = wx.ap().rearrange("p (c n) -> p c n", c=KC)
            for _q in range(KC // 2):
                _qs = slice(_q * 2, _q * 2 + 2)
                eng = nc.sync if _q % 2 == 0 else nc.scalar
                eng.dma_start(out=w_sb[:, _qs, :], in_=_wx[:, _qs, :])

        pbt_sb = const.tile([P, TT], F32)
        nc.sync.dma_start(out=pbt_sb, in_=pbt.ap())

        ts_tiles = [None] * TT
        _tst = tst.ap().rearrange("p (t x) -> p t x", t=TT)

        load_engines = [nc.gpsimd, nc.sync, nc.gpsimd, nc.scalar]

        def load(t):
            tile_ = tsp.tile([P, KC, P], TS_DT, tag="ts")
            src = _tst[:, t, :].rearrange("p (c x) -> p c x", c=KC)
            if t < 2:
                # startup: split across the SWDGE queues for parallelism
                for s in range(4):
                    cs = slice(s * (KC // 4), (s + 1) * (KC // 4))
                    nc.gpsimd.dma_start(out=tile_[:, cs, :], in_=src[:, cs, :])
            else:
                eng = load_engines[t % len(load_engines)]
                eng.dma_start(out=tile_, in_=src)
            ts_tiles[t] = tile_

        PF = 5
        for _t in range(min(PF, TT)):
            load(_t)

        deferred = {}

        def bcast(dram, parts, dt=F32):
            t = const.tile([parts, H], dt, tag=f"c_{dram.name}")
            a = dram.ap()
            nc.scalar.dma_start(
                out=t, in_=bass.AP(tensor=a.tensor, offset=a.offset,
                                   ap=[[0, parts]] + list(a.ap[1:])))
            return t

        def load_p2_consts():
            w1_sb = const.tile([P, 8, H], BF16)
            nc.scalar.dma_start(out=w1_sb,
                                in_=w1.ap().rearrange("(c p) h -> p c h", p=P))
            w2_sb = const.tile([P, 4, H], BF16)
            nc.scalar.dma_start(out=w2_sb,
                                in_=w2.ap().rearrange("(c p) h -> p c h", p=P))
            fg_sb = bcast(fg, NQ)
            fb_sb = bcast(fb, NQ)
            wo_sb = bcast(wo, NQ)
            cg_sb = const.tile([NQ, H], F32)
            nc.scalar.dma_start(out=cg_sb, in_=cg.ap())
            cb_sb = const.tile([NQ, H], F32)
            nc.scalar.dma_start(out=cb_sb, in_=cb.ap())
            bf1_sb = const.tile([1, H], BF16)
            nc.scalar.dma_start(out=bf1_sb, in_=bf1.ap())
            bf2_sb = const.tile([1, H], BF16)
            nc.scalar.dma_start(out=bf2_sb, in_=bf2.ap())
            deferred.update(w1_sb=w1_sb, w2_sb=w2_sb, fg_sb=fg_sb,
                            fb_sb=fb_sb, wo_sb=wo_sb, cg_sb=cg_sb, cb_sb=cb_sb,
                            bf1_sb=bf1_sb, bf2_sb=bf2_sb)

        ones_row = const.tile([1, P], BF16)
        nc.vector.memset(ones_row, 1.0)
        ones_col = const.tile([P, D], BF16)
        nc.vector.memset(ones_col, 1.0)
        id32 = const.tile([NQ, NQ], BF16)
        make_identity(nc, id32)
        magic_u = const.tile([P, max(TT, NQ)], U32)
        nc.vector.memset(magic_u, 0x5f3759df)
        eps_sb = const.tile([P, 1], F32)
        nc.vector.memset(eps_sb, EPS)

        # persistent activations
        x_raw = xp.tile([P, TT, H], BF16)       # un-normalized projections
        logit_sb = xp.tile([NQ, B_LOCAL], F32)

        def newton_rsqrt(out_ap, v_ap, p, n, tag):
            """out = 1/sqrt(v) elementwise, on DVE only (no ACT table).

            Bit-hack seed + 2 Newton steps: ~4e-6 rel error in fp32.
            """
            sh = lnp.tile([p, n], U32, tag=f"nw_sh_{tag}")
            nc.vector.tensor_scalar(out=sh, in0=v_ap.bitcast(U32),
                                    scalar1=1, scalar2=None,
                                    op0=mybir.AluOpType.logical_shift_right)
            y = lnp.tile([p, n], F32, tag=f"nw_y_{tag}")
            nc.vector.tensor_sub(out=y.bitcast(U32), in0=magic_u[:p, :n],
                                 in1=sh)
            t_ = lnp.tile([p, n], F32, tag=f"nw_t_{tag}")
            for it in range(2):
                nc.vector.tensor_mul(out=t_, in0=y, in1=y)
                nc.vector.scalar_tensor_tensor(
                    out=t_, in0=t_, scalar=-0.5, in1=v_ap,
                    op0=mybir.AluOpType.mult, op1=mybir.AluOpType.mult)
                nc.vector.scalar_tensor_tensor(
                    out=(y if it == 0 else out_ap), in0=t_, scalar=1.5, in1=y,
                    op0=mybir.AluOpType.add, op1=mybir.AluOpType.mult)

        def act_rsqrt(out_ap, v_ap, p, n, tag):
            s = lnp.tile([p, n], F32, tag=f"as_{tag}")
            nc.scalar.activation(out=s, in_=v_ap,
                                 func=mybir.ActivationFunctionType.Sqrt,
                                 bias=eps_sb[:p], scale=1.0)
            nc.vector.reciprocal(out=out_ap, in_=s)

        rsqrt = newton_rsqrt if NEWTON_MODE == "bithack" else act_rsqrt

        # ---------------- phase 1: project + scores + ctx per token tile ----
        ctx_ps = [None] * B_LOCAL
        expw_t = [None] * TT

        def ctx_mm(t):
            # emitted one tile late so expw(t) is ready when PE reaches it
            b, i = divmod(t, NT)
            if i == 0:
                pc = pctx.tile([NQ, H], F32, tag="pc")
                ctx_ps[b] = pc
            nc.tensor.matmul(ctx_ps[b], lhsT=expw_t[t], rhs=x_raw[:, t, :],
                             start=(i == 0), stop=(i == NT - 1))
            expw_t[t] = None

        def project(t):
            ts_sb = ts_tiles[t]
            px = psx.tile([P, H], F32, tag="px")
            sc = pssc.tile([P, NQ], F32, tag="sc")
            if FP8:
                DR = mybir.MatmulPerfMode.DoubleRow
                sc_mode = os.environ.get("KERNEL_SC_MODE", "dr")
                for p in range(KC // 2):
                    lt = ts_sb[:, 2 * p:2 * p + 2, :]
                    nc.tensor.matmul(px, lhsT=lt, rhs=w_sb[:, p, :, :H],
                                     start=(p == 0), stop=(p == KC // 2 - 1),
                                     perf_mode=DR)
                    if sc_mode == "dr":
                        nc.tensor.matmul(sc, lhsT=lt, rhs=w_sb[:, p, :, H:],
                                         start=(p == 0),
                                         stop=(p == KC // 2 - 1),
                                         perf_mode=DR)
                if sc_mode == "single":
                    for k in range(KC):
                        nc.tensor.matmul(sc, lhsT=ts_sb[:, k, :],
                                         rhs=w_sb[:, k // 2, k % 2, H:],
                                         start=(k == 0), stop=(k == KC - 1))
                elif sc_mode == "off":
                    nc.vector.memset(sc, 0.0)
            else:
                for k in range(KC):
                    nc.tensor.matmul(px, lhsT=ts_sb[:, k, :],
                                     rhs=w_sb[:, k, :H],
                                     start=(k == 0), stop=(k == KC - 1))
                    nc.tensor.matmul(sc, lhsT=ts_sb[:, k, :],
                                     rhs=w_sb[:, k, H:],
                                     start=(k == 0), stop=(k == KC - 1))
            ts_tiles[t] = None
            if t > 0:
                ctx_mm(t - 1)
            stats = lnp.tile([P, 6], F32, tag="stats")
            nc.vector.bn_stats(out=stats, in_=px)
            mv = lnp.tile([P, 2], F32, tag="mv")
            nc.vector.bn_aggr(out=mv, in_=stats)
            nc.scalar.copy(out=x_raw[:, t, :], in_=px)
            # rstd for this tile's tokens (per-token over H -> tile-local)
            v = lnp.tile([P, 1], F32, tag="ptv")
            nc.vector.tensor_scalar_add(out=v, in0=mv[:, 1:2], scalar1=EPS)
            rstd = lnp.tile([P, 1], F32, tag="ptr")
            rsqrt(rstd, v, P, 1, "p1")
            # unnormalized softmax numerator, rstd folded for the ctx trick
            sexp = lnp.tile([P, NQ], F32, tag="sexp")
            nc.scalar.activation(out=sexp, in_=sc,
                                 func=mybir.ActivationFunctionType.Exp,
                                 bias=pbt_sb[:, t:t + 1], scale=rstd)
            expw = lnp.tile([P, NQ], BF16, tag="expw")
            nc.vector.tensor_scalar_mul(out=expw, in0=sexp, scalar1=rstd)
            expw_t[t] = expw

        # ---------------- phase 2 per batch row ------------------------------
        st2 = {}

        def p2_ctxln(b):
            st2[b] = {}
            pc = ctx_ps[b]
            stats = lnp.tile([NQ, 6], F32, tag="stats2")
            nc.vector.bn_stats(out=stats, in_=pc)
            mv = lnp.tile([NQ, 2], F32, tag="mv2")
            nc.vector.bn_aggr(out=mv, in_=stats)
            rstd = lnp.tile([NQ, 1], F32, tag="rstd2")
            v2 = lnp.tile([NQ, 1], F32, tag="v2c")
            if NEWTON_MODE == "bithack":
                nc.vector.tensor_scalar_add(out=v2, in0=mv[:, 1:2], scalar1=EPS)
            else:
                nc.vector.tensor_copy(out=v2, in_=mv[:, 1:2])
            rsqrt(rstd, v2, NQ, 1, "p2c")
            ctxln = p2.tile([NQ, H], BF16, tag="ctxln")
            if "gcln" in skip:
                nc.vector.tensor_scalar(out=ctxln, in0=pc, scalar1=mv[:, 0:1],
                                        scalar2=rstd,
                                        op0=mybir.AluOpType.subtract,
                                        op1=mybir.AluOpType.mult)
            else:
                cn = p2.tile([NQ, H], F32, tag="cn")
                nc.vector.tensor_scalar(out=cn, in0=pc, scalar1=mv[:, 0:1],
                                        scalar2=rstd,
                                        op0=mybir.AluOpType.subtract,
                                        op1=mybir.AluOpType.mult)
                cgn = p2.tile([NQ, H], F32, tag="cgn")
                nc.vector.tensor_mul(out=cgn, in0=cn, in1=deferred["cg_sb"])
                nc.vector.tensor_add(out=ctxln, in0=cgn, in1=deferred["cb_sb"])
            st2[b]["ctxln"] = ctxln

        def p2_ctxT(b):
            ctxln = st2[b]["ctxln"]
            pct = pst.tile([P, 4 * NQ], BF16, tag="ps2t")
            for j in range(4):
                nc.tensor.transpose(pct[:, j * NQ:(j + 1) * NQ],
                                    ctxln[:, j * P:(j + 1) * P], id32)
            ctxT = p2.tile([P, 4, NQ], BF16, tag="ctxT")
            nc.vector.tensor_copy(out=ctxT, in_=pct)
            gcol = p2.tile([P, 4, 1], F32, tag="gcol")
            nc.vector.tensor_copy(
                out=gcol, in_=pct.rearrange("p (c q) -> p c q", q=NQ)[:, :, 0:1])

            # fused^T [128, 8, 16]: chunks 0-3 = d_ctx^T, 4-7 = g_ctx^T bcast
            fusedT = p2.tile([P, 8, D], BF16, tag="fusedT")
            for c in range(4):
                nc.vector.tensor_copy(out=fusedT[:, c, :], in_=ctxT[:, c, 1:1 + D])
            for c in range(4):
                nc.vector.tensor_scalar_mul(out=fusedT[:, 4 + c, :], in0=ones_col,
                                            scalar1=gcol[:, c, :])
            st2[b]["fusedT"] = fusedT

        def p2_ffn(b):
            fusedT = st2[b]["fusedT"]
            ph1 = pffn.tile([NQ, H], F32, tag="ps_ffn")
            for kc in range(8):
                nc.tensor.matmul(ph1[:D, :], lhsT=fusedT[:, kc, :],
                                 rhs=deferred["w1_sb"][:, kc, :], start=(kc == 0),
                                 stop=(kc == 7 and "bf1" in skip))
            if "bf1" not in skip:
                nc.tensor.matmul(ph1[:D, :], lhsT=ones_row[:, :D],
                                 rhs=deferred["bf1_sb"], start=False, stop=True)
            h1 = p2.tile([NQ, H], F32, tag="h1")
            nc.scalar.activation(out=h1[:D, :], in_=ph1[:D, :],
                                 func=mybir.ActivationFunctionType.Gelu)

            # fln LN
            stats = lnp.tile([NQ, 6], F32, tag="stats2")
            nc.vector.bn_stats(out=stats[:D, :], in_=h1[:D, :])
            mv = lnp.tile([NQ, 2], F32, tag="mv2")
            nc.vector.bn_aggr(out=mv[:D, :], in_=stats[:D, :])
            rstd = lnp.tile([NQ, 1], F32, tag="rstd2")
            v2 = lnp.tile([NQ, 1], F32, tag="v2f")
            if NEWTON_MODE == "bithack":
                nc.vector.tensor_scalar_add(out=v2[:D, :], in0=mv[:D, 1:2],
                                            scalar1=EPS)
            else:
                nc.vector.tensor_copy(out=v2[:D, :], in_=mv[:D, 1:2])
            rsqrt(rstd[:D, :], v2[:D, :], D, 1, "p2f")
            h1ln = p2.tile([NQ, H], BF16, tag="h1ln")
            if "fln" in skip:
                nc.vector.tensor_scalar(out=h1ln[:D, :], in0=h1[:D, :],
                                        scalar1=mv[:D, 0:1], scalar2=rstd[:D, :],
                                        op0=mybir.AluOpType.subtract,
                                        op1=mybir.AluOpType.mult)
            else:
                h1n = p2.tile([NQ, H], F32, tag="h1n")
                nc.vector.tensor_scalar(out=h1n[:D, :], in0=h1[:D, :],
                                        scalar1=mv[:D, 0:1], scalar2=rstd[:D, :],
                                        op0=mybir.AluOpType.subtract,
                                        op1=mybir.AluOpType.mult)
                h1g = p2.tile([NQ, H], F32, tag="h1g")
                nc.vector.tensor_mul(out=h1g[:D, :], in0=h1n[:D, :],
                                     in1=deferred["fg_sb"][:D, :])
                nc.vector.tensor_add(out=h1ln[:D, :], in0=h1g[:D, :],
                                     in1=deferred["fb_sb"][:D, :])
            st2[b]["h1ln"] = h1ln

        def p2_ffn2(b):
            h1ln = st2[b]["h1ln"]
            ph1t = pst.tile([P, 4 * NQ], BF16, tag="ps2t")
            for j in range(4):
                nc.tensor.transpose(ph1t[:, j * D:(j + 1) * D],
                                    h1ln[:D, j * P:(j + 1) * P], id32[:D, :D])
            h1T = p2.tile([P, 4, D], BF16, tag="h1T")
            nc.vector.tensor_copy(out=h1T, in_=ph1t[:, :4 * D])

            ph2 = pffn.tile([NQ, H], F32, tag="ps_ffn")
            for kc in range(4):
                nc.tensor.matmul(ph2[:D, :], lhsT=h1T[:, kc, :],
                                 rhs=deferred["w2_sb"][:, kc, :], start=(kc == 0),
                                 stop=(kc == 3 and "bf2" in skip))
            if "bf2" not in skip:
                nc.tensor.matmul(ph2[:D, :], lhsT=ones_row[:, :D],
                                 rhs=deferred["bf2_sb"], start=False, stop=True)
            h2 = p2.tile([NQ, H], F32, tag="h2")
            nc.scalar.activation(out=h2[:D, :], in_=ph2[:D, :],
                                 func=mybir.ActivationFunctionType.Gelu)

            # logits = h2 . wo + b_out_s   (wo has 1/temperature folded in)
            prod = p2.tile([NQ, H], F32, tag="prod")
            nc.vector.tensor_mul(out=prod[:D, :], in0=h2[:D, :],
                                 in1=deferred["wo_sb"][:D, :])
            lsum = lnp.tile([NQ, 1], F32, tag="lsum")
            nc.vector.reduce_sum(out=lsum[:D, :], in_=prod[:D, :],
                                 axis=mybir.AxisListType.X)
            nc.vector.tensor_scalar_add(out=logit_sb[:D, b:b + 1],
                                        in0=lsum[:D, :], scalar1=float(b_out_s))

        # driver: pipeline tiles; spread batch b's phase 2 across the next
        # batch's projection stream so PE never starves.
        p2_stages = [p2_ctxln, p2_ctxT, p2_ffn, p2_ffn2]
        stage = [0] * B_LOCAL
        _outT = out.ap().rearrange("b d -> d b")

        def advance(b):
            p2_stages[stage[b]](b)
            stage[b] += 1
            if stage[b] == len(p2_stages):
                nc.sync.dma_start(out=_outT[:, b:b + 1],
                                  in_=logit_sb[:D, b:b + 1])

        for t in range(TT):
            if t + PF < TT:
                load(t + PF)
            project(t)
            if t == 0:
                load_p2_consts()
            done = t  # tiles fully projected up to here
            for b in range(B_LOCAL - 1):
                if done >= (b + 1) * NT + stage[b] and stage[b] < len(p2_stages):
                    advance(b)
        ctx_mm(TT - 1)
        for b in range(B_LOCAL):
            while stage[b] < len(p2_stages):
                advance(b)


def _np(x):
    return np.asarray(x)


LAST_RESULT = None


def kernel(**inputs):
    from concourse.bass_utils import run_bass_kernel_spmd

    token_states = _np(inputs["token_states"]).astype(np.float32)
    mask = _np(inputs["attention_mask"])
    W_proj = _np(inputs["W_proj"]).astype(np.float64)
    b_proj = _np(inputs["b_proj"]).astype(np.float64)
    tln_g = _np(inputs["tln_g"]).astype(np.float64)
    tln_b = _np(inputs["tln_b"]).astype(np.float64)
    gln_g = _np(inputs["gln_g"]).astype(np.float32)
    gln_b = _np(inputs["gln_b"]).astype(np.float32)
    cln_g = _np(inputs["cln_g"]).astype(np.float32)
    cln_b = _np(inputs["cln_b"]).astype(np.float32)
    fln_g = _np(inputs["fln_g"]).astype(np.float32)
    fln_b = _np(inputs["fln_b"]).astype(np.float32)
    domain_queries = _np(inputs["domain_queries"]).astype(np.float64)
    global_query = _np(inputs["global_query"]).astype(np.float64)
    W_ff1 = _np(inputs["W_ff1"]).astype(np.float32)
    b_ff1 = _np(inputs["b_ff1"]).astype(np.float32)
    W_ff2 = _np(inputs["W_ff2"]).astype(np.float32)
    b_ff2 = _np(inputs["b_ff2"]).astype(np.float32)
    W_out = _np(inputs["W_out"]).astype(np.float32)
    b_out = _np(inputs["b_out"]).astype(np.float32)
    log_temperature = _np(inputs["log_temperature"]).astype(np.float32)

    Bq, Sq = mask.shape
    assert (Bq, Sq) == (B, S) and token_states.shape == (B, S, TOKD)

    # v2 fast path requires the token LN to be a pure normalization and a
    # zero projection bias (holds for this problem's inputs); the scores
    # fold and the un-normalized context trick rely on it.
    assert np.all(tln_b == 0) and np.all(b_proj == 0) \
        and np.all(tln_g == tln_g[0]), \
        "kernel v2 requires tln_b == 0, b_proj == 0, constant tln_g"

    # ---- host preprocessing ----
    counts = mask.astype(bool).sum(axis=1)
    S_c = int(max(128, -(-int(counts.max()) // P) * P))
    NT = S_c // P
    TT = B_LOCAL * NT

    bf16 = ml_dtypes.bfloat16
    ts_np = mybir.dt.np(FP8DT) if FP8 else bf16

    ts_c = np.zeros((B, S_c, TOKD), ts_np)
    padbias = np.full((B, S_c), -1e9, np.float32)
    for b in range(B):
        idx = np.flatnonzero(mask[b])
        n = len(idx)
        ts_c[b, :n] = token_states[b, idx]
        padbias[b, :n] = 0.0

    temp = float(np.clip(np.exp(log_temperature[0]), 0.3, 3.0))
    inv_t = 1.0 / temp
    wo_host = (W_out[:, 0] * inv_t).astype(np.float32)
    b_out_s = float(b_out[0] * inv_t)

    # scores fold: WM = W @ (g*q) - (W @ 1_H) * (sum(g*q)/H)
    q_all = np.concatenate([global_query[None, :], domain_queries], axis=0)  # [17,H]
    gq = (tln_g[None, :] * q_all).T                      # [H, 17]
    gq_pad = np.zeros((H, NQ), np.float64)
    gq_pad[:, :17] = gq
    sgq = gq_pad.sum(axis=0)                             # [32]
    Wgq = W_proj @ gq_pad                                # [4096, 32]
    Wrow = (W_proj @ np.ones((H, 1))) / H                # [4096, 1]
    WM = Wgq - Wrow * sgq[None, :]                       # [4096, 32]

    W_ext = np.zeros((TOKD, NS), np.float64)
    W_ext[:, :H] = W_proj
    W_ext[:, H:] = WM
    W_ext *= WSCALE
    if FP8:
        # DoubleRow rhs layout [p, pair, j, n]
        wx_host = np.ascontiguousarray(
            W_ext.reshape(KC // 2, 2, P, NS).transpose(2, 0, 1, 3)
        ).astype(ts_np).reshape(P, KC * NS)
    else:
        wx_host = np.ascontiguousarray(
            W_ext.reshape(KC, P, NS).transpose(1, 0, 2)
        ).astype(ts_np).reshape(P, KC * NS)

    cg_host = np.ones((NQ, H), np.float32)
    cb_host = np.zeros((NQ, H), np.float32)
    cg_host[0] = gln_g
    cb_host[0] = gln_b
    cg_host[1:17] = cln_g
    cb_host[1:17] = cln_b

    skip = set()
    if np.all(cg_host == 1) and np.all(cb_host == 0):
        skip.add("gcln")
    if np.all(fln_g == 1) and np.all(fln_b == 0):
        skip.add("fln")
    if np.all(b_ff1 == 0):
        skip.add("bf1")
    if np.all(b_ff2 == 0):
        skip.add("bf2")

    nc = build_nc(S_c, b_out_s, frozenset(skip))

    shared = dict(
        wx=wx_host,
        cg=cg_host, cb=cb_host,
        fg=fln_g[None, :], fb=fln_b[None, :],
        w1=W_ff1.astype(bf16), bf1=b_ff1[None, :].astype(bf16),
        w2=W_ff2.astype(bf16), bf2=b_ff2[None, :].astype(bf16),
        wo=wo_host[None, :],
    )

    in_maps = []
    for c in range(N_CORES):
        m = dict(shared)
        bs = slice(c * B_LOCAL, (c + 1) * B_LOCAL)
        # [B_LOCAL*S_c, TOKD] -> [p, t, c, tok] pre-transposed bf16
        tsl = ts_c[bs].reshape(TT, P, KC, P)
        m["tst"] = np.ascontiguousarray(
            tsl.transpose(3, 0, 2, 1)).reshape(P, TT * KC * P)
        m["pbt"] = np.ascontiguousarray(
            padbias[bs].reshape(TT, P).T)
        in_maps.append(m)

    trace = os.environ.get("KERNEL_TRACE", "0") == "1"
    kw = {}
    if trace:
        kw = dict(trace=True, tmpdir=os.environ.get("KERNEL_TRACE_DIR") or None)
    res = run_bass_kernel_spmd(nc, in_maps, core_ids=list(range(N_CORES)), **kw)
    global LAST_RESULT
    LAST_RESULT = res
    outs = [res.results[c]["out"] for c in range(N_CORES)]
    return np.concatenate(outs, axis=0).astype(np.float32)


if __name__ == "__main__":
    pass
